# revision 23
# baseline (speedup 1.0000x reference)
"""Trainium2 Bass kernel for the Low-Rank TBRNN problem.

Math (per batch row, fp32):
    x_{t+1} = x_t + std*noise_t + tau*(-x_t + rec_t + inp_t)
            = a*x_t + d_t + kappa*((r_t@M) o (r_t@N)) @ L^T
  with a = 1-tau, d_t = tau*(u_t@W_in^T + b_in) + std*noise_t,
  kappa = tau/(H*H), r_t = tanh(x_t).

Because kappa ~ 7.6e-7, the bilinear term is a tiny perturbation of an
otherwise *linear* recurrence.  We exploit the exact first-order
decomposition
    x_t = x0scan_t + (L y_t),   y_{t+1} = a*y_t + kappa*c_t,
    c_t = (r_t@Mk) o (r_t@N),   r_t = tanh(x0scan_t)    [first order]
whose error vs the exact recurrence is ~1e-11 (validated numerically,
far below fp32 rounding of the reference itself).  Everything is then
bulk work: two IIR scans (DVE tensor_tensor_scan), big matmuls, and
elementwise passes -- memory-bound instead of latency-bound.

Sharding: data-parallel over batch B=64 across 8 cores (B'=8 per core);
small parameters replicated; the full time axis stays local per core.
"""

import sys

sys.path.insert(0, "/opt/trn_rl_repo")

import numpy as np
import ml_dtypes

import concourse.bacc as bacc
import concourse.bass as bass
import concourse.mybir as mybir
import concourse.tile as tile
from concourse.bass_utils import run_bass_kernel_spmd
from concourse.mybir import AluOpType

F32 = mybir.dt.float32
BF16 = mybir.dt.bfloat16
TANH = mybir.ActivationFunctionType.Tanh

# problem constants (hardcoded per spec)
B, T, I, H, R, O = 64, 1024, 64, 512, 8, 64
NCORES = 8
BP = B // NCORES          # batch per core = 8
P = 128                   # partitions
NHC = H // P              # h chunks = 4
TC = 512                  # time/column chunk for fp32 matmuls
NOISE_STD = 0.05
TAU = 0.2
SCALE = 1.0 / (H * H)
ALPHA = 1.0 - TAU
KAPPA = TAU * SCALE


def _build_nc():
    nc = bacc.Bacc("TRN2", target_bir_lowering=False, debug=False)

    # per-core inputs (host pre-transposed so every DMA is contiguous)
    u = nc.dram_tensor("u", [BP, I, T], F32, kind="ExternalInput")
    noise = nc.dram_tensor("noise", [BP, H, T], F32, kind="ExternalInput")
    x0t = nc.dram_tensor("x0t", [H, BP], F32, kind="ExternalInput")
    wint = nc.dram_tensor("wint", [I + 1, H], F32, kind="ExternalInput")
    # [kappa*M | zeros | N]: a lands on partitions 0-7, b on 32-39 (32-aligned
    # bases are required for engine partition access)
    mn = nc.dram_tensor("mn", [H, 40], BF16, kind="ExternalInput")
    lt = nc.dram_tensor("lt", [R, H], F32, kind="ExternalInput")
    woutt = nc.dram_tensor("woutt", [H, O], F32, kind="ExternalInput")
    bout = nc.dram_tensor("bout", [1, O], F32, kind="ExternalInput")
    identstd = nc.dram_tensor("identstd", [P, P], F32, kind="ExternalInput")

    # per-core outputs ([.., T]-minor layouts; host transposes back)
    outp = nc.dram_tensor("outp", [BP, O, T], F32, kind="ExternalOutput")
    xlast = nc.dram_tensor("xlast", [BP, H], F32, kind="ExternalOutput")
    traj = nc.dram_tensor("traj", [BP, H, T], F32, kind="ExternalOutput")

    with tile.TileContext(nc) as tc:
        with (
            tc.tile_pool(name="singles", bufs=1) as singles,
            tc.tile_pool(name="noisep", bufs=3) as noisep,
            tc.tile_pool(name="xp", bufs=8) as xp,
            tc.tile_pool(name="r0p", bufs=6) as r0p,
            tc.tile_pool(name="abp", bufs=2) as abp,
            tc.tile_pool(name="cyp", bufs=4) as cyp,
            tc.tile_pool(name="r1p", bufs=10) as r1p,
            tc.tile_pool(name="trp", bufs=8) as trp,
            tc.tile_pool(name="otp", bufs=4) as otp,
            tc.tile_pool(name="psd", bufs=2, space="PSUM") as psd,
            tc.tile_pool(name="psab", bufs=2, space="PSUM") as psab,
            tc.tile_pool(name="pst", bufs=2, space="PSUM") as pst,
            tc.tile_pool(name="pso", bufs=2, space="PSUM") as pso,
        ):
            # ---- constants resident in SBUF ----
            alpha_sb = singles.tile([P, T], F32)
            nc.gpsimd.memset(alpha_sb[:], ALPHA)
            ones_sb = singles.tile([1, TC], F32)
            nc.gpsimd.memset(ones_sb[:], 1.0)
            wint_sb = singles.tile([I + 1, H], F32)
            nc.sync.dma_start(out=wint_sb[:], in_=wint[:])
            mn_sb = singles.tile([P, NHC, 40], BF16)
            nc.sync.dma_start(out=mn_sb[:], in_=mn.rearrange("(c p) j -> p c j", p=P))
            lt_sb = singles.tile([R, H], F32)
            nc.sync.dma_start(out=lt_sb[:], in_=lt[:])
            woutt_sb = singles.tile([P, NHC, O], F32)
            nc.sync.dma_start(
                out=woutt_sb[:], in_=woutt.rearrange("(c p) j -> p c j", p=P)
            )
            bout_sb = singles.tile([1, O], F32)
            nc.sync.dma_start(out=bout_sb[:], in_=bout[:])
            identstd_sb = singles.tile([P, P], F32)
            nc.sync.dma_start(out=identstd_sb[:], in_=identstd[:])
            x0t_sb = singles.tile([P, NHC, BP], F32)
            nc.sync.dma_start(out=x0t_sb[:], in_=x0t.rearrange("(c p) b -> p c b", p=P))
            # two persistent u^T buffers (row I is the all-ones bias row)
            ut_bufs = []
            for k in range(2):
                ub = singles.tile([I + 1, T], F32, tag=f"ut{k}")
                nc.gpsimd.memset(ub[I : I + 1, :], 1.0)
                ut_bufs.append(ub)

            for b in range(BP):
                ut = ut_bufs[b % 2]
                nc.sync.dma_start(out=ut[0:I, :], in_=u[b])

                xts = []
                r0s = []
                for hc in range(NHC):
                    hsl = slice(hc * P, (hc + 1) * P)
                    noiset = noisep.tile([P, T], F32)
                    nc.sync.dma_start(out=noiset[:], in_=noise[b, hsl, :])
                    xt = xp.tile([P, T], F32)
                    for tci in range(T // TC):
                        tsl = slice(tci * TC, (tci + 1) * TC)
                        pd = psd.tile([P, TC], F32)
                        # d = tau*(W_in u + b_in) + std*noise, accumulated in PSUM
                        nc.tensor.matmul(
                            pd[:], wint_sb[:, hsl], ut[:, tsl], start=True, stop=False
                        )
                        nc.tensor.matmul(
                            pd[:], identstd_sb[:], noiset[:, tsl], start=False, stop=True
                        )
                        # x0scan_{t+1} = alpha*x + d_t   (IIR scan along t)
                        init = (
                            x0t_sb[:, hc, b : b + 1]
                            if tci == 0
                            else xt[:, tci * TC - 1 : tci * TC]
                        )
                        nc.vector.tensor_tensor_scan(
                            xt[:, tsl],
                            alpha_sb[:, 0:TC],
                            pd[:],
                            init,
                            AluOpType.mult,
                            AluOpType.add,
                        )
                    r0 = r0p.tile([P, T], BF16)
                    nc.scalar.activation(r0[:], xt[:], TANH)
                    xts.append(xt)
                    r0s.append(r0)

                # a|b = [kappa*M | N]^T r0   (contract over H)
                ab_b = abp.tile([R, T], F32)
                c_sb = cyp.tile([R, T], F32, tag="c")
                for tci in range(T // TC):
                    tsl = slice(tci * TC, (tci + 1) * TC)
                    pab = psab.tile([40, TC], F32)
                    for hc in range(NHC):
                        nc.tensor.matmul(
                            pab[:],
                            mn_sb[:, hc, :],
                            r0s[hc][:, tsl],
                            start=(hc == 0),
                            stop=(hc == NHC - 1),
                        )
                    nc.scalar.copy(ab_b[:, tsl], pab[32:40, :])
                    nc.vector.tensor_tensor(
                        c_sb[:, tsl], pab[0:R, :], ab_b[:, tsl], AluOpType.mult
                    )
                # y_{t+1} = alpha*y + c_t  (tiny R-space IIR scan)
                y_sb = cyp.tile([R, T], F32, tag="y")
                nc.vector.tensor_tensor_scan(
                    y_sb[:],
                    alpha_sb[0:R, :],
                    c_sb[:],
                    0.0,
                    AluOpType.mult,
                    AluOpType.add,
                )

                # traj = x0scan + L y ; out = tanh(traj) @ W_out^T + b_out
                for tci in range(T // TC):
                    tsl = slice(tci * TC, (tci + 1) * TC)
                    r1s = []
                    for hc in range(NHC):
                        hsl = slice(hc * P, (hc + 1) * P)
                        pt = pst.tile([P, TC], F32)
                        nc.tensor.matmul(
                            pt[:], lt_sb[:, hsl], y_sb[:, tsl], start=True, stop=True
                        )
                        tr = trp.tile([P, TC], F32)
                        nc.vector.tensor_tensor(
                            tr[:], pt[:], xts[hc][:, tsl], AluOpType.add
                        )
                        nc.sync.dma_start(out=traj[b, hsl, tsl], in_=tr[:])
                        if tci == T // TC - 1:
                            nc.sync.dma_start(
                                out=xlast[b, hsl].rearrange("(h o) -> h o", o=1),
                                in_=tr[:, TC - 1 : TC],
                            )
                        r1 = r1p.tile([P, TC], F32)
                        nc.scalar.activation(r1[:], tr[:], TANH)
                        r1s.append(r1)
                    po = pso.tile([O, TC], F32)
                    nc.tensor.matmul(
                        po[:], bout_sb[:], ones_sb[:], start=True, stop=False
                    )
                    for hc in range(NHC):
                        nc.tensor.matmul(
                            po[:],
                            woutt_sb[:, hc, :],
                            r1s[hc][:],
                            start=False,
                            stop=(hc == NHC - 1),
                        )
                    ot = otp.tile([O, TC], F32)
                    nc.scalar.copy(ot[:], po[:])
                    nc.sync.dma_start(out=outp[b, :, tsl], in_=ot[:])

    nc.finalize()
    return nc


_NC_CACHE = None
_LAST_EXEC_NS = None
_LAST_RESULTS = None


def _get_nc():
    global _NC_CACHE
    if _NC_CACHE is None:
        _NC_CACHE = _build_nc()
    return _NC_CACHE


def kernel(u, x0, noise, L, M, N, W_in, b_in, W_out, b_out):
    u = np.asarray(u, np.float32)
    x0 = np.asarray(x0, np.float32)
    noise = np.asarray(noise, np.float32)
    L = np.asarray(L, np.float32)
    M = np.asarray(M, np.float32)
    N = np.asarray(N, np.float32)
    W_in = np.asarray(W_in, np.float32)
    b_in = np.asarray(b_in, np.float32)
    W_out = np.asarray(W_out, np.float32)
    b_out = np.asarray(b_out, np.float32)

    # host-side constant prep (replicated across cores)
    wint = np.concatenate(
        [(TAU * W_in).T.astype(np.float32), (TAU * b_in)[None, :].astype(np.float32)],
        axis=0,
    )  # [I+1, H]
    mn = np.zeros((H, 40), dtype=ml_dtypes.bfloat16)
    mn[:, 0:R] = (KAPPA * M).astype(ml_dtypes.bfloat16)
    mn[:, 32 : 32 + R] = N.astype(ml_dtypes.bfloat16)
    lt = np.ascontiguousarray(L.T).astype(np.float32)  # [R, H]
    woutt = np.ascontiguousarray(W_out.T).astype(np.float32)  # [H, O]
    boutr = np.ascontiguousarray(b_out[None, :]).astype(np.float32)  # [1, O]
    identstd = (NOISE_STD * np.eye(P)).astype(np.float32)

    in_maps = []
    for k in range(NCORES):
        bsl = slice(k * BP, (k + 1) * BP)
        in_maps.append(
            {
                "u": np.ascontiguousarray(u[bsl].transpose(0, 2, 1)),
                "noise": np.ascontiguousarray(noise[:, bsl, :].transpose(1, 2, 0)),
                "x0t": np.ascontiguousarray(x0[bsl].T),
                "wint": wint,
                "mn": mn,
                "lt": lt,
                "woutt": woutt,
                "bout": boutr,
                "identstd": identstd,
            }
        )

    nc = _get_nc()
    import os

    trace = bool(int(os.environ.get("KERNEL_TRACE", "0")))
    res = run_bass_kernel_spmd(nc, in_maps, core_ids=list(range(NCORES)), trace=trace)
    global _LAST_EXEC_NS, _LAST_RESULTS
    _LAST_EXEC_NS = res.exec_time_ns
    _LAST_RESULTS = res

    out = np.ascontiguousarray(
        np.concatenate([res.results[k]["outp"] for k in range(NCORES)], axis=0)
        .transpose(0, 2, 1)
    )
    x_last = np.concatenate([res.results[k]["xlast"] for k in range(NCORES)], axis=0)
    trj = np.ascontiguousarray(
        np.concatenate([res.results[k]["traj"] for k in range(NCORES)], axis=0)
        .transpose(0, 2, 1)
    )
    return out, x_last, trj


# revision 24
# speedup vs baseline: 1.0897x; 1.0897x over previous
"""Trainium2 Bass kernel for the Low-Rank TBRNN problem.

Math (per batch row, fp32):
    x_{t+1} = x_t + std*noise_t + tau*(-x_t + rec_t + inp_t)
            = a*x_t + d_t + kappa*((r_t@M) o (r_t@N)) @ L^T
  with a = 1-tau, d_t = tau*(u_t@W_in^T + b_in) + std*noise_t,
  kappa = tau/(H*H), r_t = tanh(x_t).

Because kappa ~ 7.6e-7, the bilinear term is a tiny perturbation of an
otherwise *linear* recurrence.  We exploit the exact first-order
decomposition
    x_t = x0scan_t + (L y_t),   y_{t+1} = a*y_t + kappa*c_t,
    c_t = (r_t@Mk) o (r_t@N),   r_t = tanh(x0scan_t)    [first order]
whose error vs the exact recurrence is ~1e-11 (validated numerically,
far below fp32 rounding of the reference itself).  Everything is then
bulk work: two IIR scans (DVE tensor_tensor_scan), big matmuls, and
elementwise passes -- memory-bound instead of latency-bound.

Sharding: data-parallel over batch B=64 across 8 cores (B'=8 per core);
small parameters replicated; the full time axis stays local per core.
"""

import sys

sys.path.insert(0, "/opt/trn_rl_repo")

import numpy as np
import ml_dtypes

import concourse.bacc as bacc
import concourse.bass as bass
import concourse.mybir as mybir
import concourse.tile as tile
from concourse.bass_utils import run_bass_kernel_spmd
from concourse.mybir import AluOpType

F32 = mybir.dt.float32
BF16 = mybir.dt.bfloat16
TANH = mybir.ActivationFunctionType.Tanh

# problem constants (hardcoded per spec)
B, T, I, H, R, O = 64, 1024, 64, 512, 8, 64
NCORES = 8
BP = B // NCORES          # batch per core = 8
P = 128                   # partitions
NHC = H // P              # h chunks = 4
TC = 512                  # time/column chunk for fp32 matmuls
NOISE_STD = 0.05
TAU = 0.2
SCALE = 1.0 / (H * H)
ALPHA = 1.0 - TAU
KAPPA = TAU * SCALE


def _build_nc():
    nc = bacc.Bacc("TRN2", target_bir_lowering=False, debug=False)

    # per-core inputs (host pre-transposed so every DMA is contiguous)
    u = nc.dram_tensor("u", [BP, I, T], F32, kind="ExternalInput")
    noise = nc.dram_tensor("noise", [BP, H, T], F32, kind="ExternalInput")
    x0t = nc.dram_tensor("x0t", [H, BP], F32, kind="ExternalInput")
    wint = nc.dram_tensor("wint", [I + 1, H], F32, kind="ExternalInput")
    # [kappa*M | zeros | N]: a lands on partitions 0-7, b on 32-39 (32-aligned
    # bases are required for engine partition access)
    mn = nc.dram_tensor("mn", [H, 40], BF16, kind="ExternalInput")
    lt = nc.dram_tensor("lt", [R, H], F32, kind="ExternalInput")
    woutt = nc.dram_tensor("woutt", [H, O], F32, kind="ExternalInput")
    bout = nc.dram_tensor("bout", [1, O], F32, kind="ExternalInput")
    identstd = nc.dram_tensor("identstd", [P, P], F32, kind="ExternalInput")

    # per-core outputs ([.., T]-minor layouts; host transposes back)
    outp = nc.dram_tensor("outp", [BP, O, T], F32, kind="ExternalOutput")
    xlast = nc.dram_tensor("xlast", [BP, H], F32, kind="ExternalOutput")
    traj = nc.dram_tensor("traj", [BP, H, T], F32, kind="ExternalOutput")

    with tile.TileContext(nc) as tc:
        with (
            tc.tile_pool(name="singles", bufs=1) as singles,
            tc.tile_pool(name="noisep", bufs=5) as noisep,
            tc.tile_pool(name="xp", bufs=9) as xp,
            tc.tile_pool(name="r0p", bufs=8) as r0p,
            tc.tile_pool(name="abp", bufs=3) as abp,
            tc.tile_pool(name="cyp", bufs=4) as cyp,
            tc.tile_pool(name="r1p", bufs=12) as r1p,
            tc.tile_pool(name="trp", bufs=10) as trp,
            tc.tile_pool(name="otp", bufs=4) as otp,
            tc.tile_pool(name="psd", bufs=2, space="PSUM") as psd,
            tc.tile_pool(name="psab", bufs=2, space="PSUM") as psab,
            tc.tile_pool(name="pst", bufs=2, space="PSUM") as pst,
            tc.tile_pool(name="pso", bufs=2, space="PSUM") as pso,
        ):
            # ---- constants resident in SBUF ----
            alpha_sb = singles.tile([P, T], F32)
            nc.gpsimd.memset(alpha_sb[:], ALPHA)
            ones_sb = singles.tile([1, TC], F32)
            nc.gpsimd.memset(ones_sb[:], 1.0)
            wint_sb = singles.tile([I + 1, H], F32)
            nc.sync.dma_start(out=wint_sb[:], in_=wint[:])
            mn_sb = singles.tile([P, NHC, 40], BF16)
            nc.sync.dma_start(out=mn_sb[:], in_=mn.rearrange("(c p) j -> p c j", p=P))
            lt_sb = singles.tile([R, H], F32)
            nc.sync.dma_start(out=lt_sb[:], in_=lt[:])
            woutt_sb = singles.tile([P, NHC, O], F32)
            nc.sync.dma_start(
                out=woutt_sb[:], in_=woutt.rearrange("(c p) j -> p c j", p=P)
            )
            bout_sb = singles.tile([1, O], F32)
            nc.sync.dma_start(out=bout_sb[:], in_=bout[:])
            identstd_sb = singles.tile([P, P], F32)
            nc.sync.dma_start(out=identstd_sb[:], in_=identstd[:])
            x0t_sb = singles.tile([P, NHC, BP], F32)
            nc.sync.dma_start(out=x0t_sb[:], in_=x0t.rearrange("(c p) b -> p c b", p=P))
            # two persistent u^T buffers (row I is the all-ones bias row)
            ut_bufs = []
            for k in range(2):
                ub = singles.tile([I + 1, T], F32, tag=f"ut{k}")
                nc.gpsimd.memset(ub[I : I + 1, :], 1.0)
                ut_bufs.append(ub)

            for b in range(BP):
                ut = ut_bufs[b % 2]
                nc.sync.dma_start(out=ut[0:I, :], in_=u[b])

                xts = []
                r0s = []
                for hc in range(NHC):
                    hsl = slice(hc * P, (hc + 1) * P)
                    noiset = noisep.tile([P, T], F32)
                    nc.sync.dma_start(out=noiset[:], in_=noise[b, hsl, :])
                    xt = xp.tile([P, T], F32)
                    for tci in range(T // TC):
                        tsl = slice(tci * TC, (tci + 1) * TC)
                        pd = psd.tile([P, TC], F32)
                        # d = tau*(W_in u + b_in) + std*noise, accumulated in PSUM
                        nc.tensor.matmul(
                            pd[:], wint_sb[:, hsl], ut[:, tsl], start=True, stop=False
                        )
                        nc.tensor.matmul(
                            pd[:], identstd_sb[:], noiset[:, tsl], start=False, stop=True
                        )
                        # x0scan_{t+1} = alpha*x + d_t   (IIR scan along t)
                        init = (
                            x0t_sb[:, hc, b : b + 1]
                            if tci == 0
                            else xt[:, tci * TC - 1 : tci * TC]
                        )
                        nc.vector.tensor_tensor_scan(
                            xt[:, tsl],
                            alpha_sb[:, 0:TC],
                            pd[:],
                            init,
                            AluOpType.mult,
                            AluOpType.add,
                        )
                    r0 = r0p.tile([P, T], BF16)
                    nc.scalar.activation(r0[:], xt[:], TANH)
                    xts.append(xt)
                    r0s.append(r0)

                # a|b = [kappa*M | N]^T r0   (contract over H)
                ab_b = abp.tile([R, T], F32)
                c_sb = cyp.tile([R, T], F32, tag="c")
                for tci in range(T // TC):
                    tsl = slice(tci * TC, (tci + 1) * TC)
                    pab = psab.tile([40, TC], F32)
                    for hc in range(NHC):
                        nc.tensor.matmul(
                            pab[:],
                            mn_sb[:, hc, :],
                            r0s[hc][:, tsl],
                            start=(hc == 0),
                            stop=(hc == NHC - 1),
                        )
                    nc.scalar.copy(ab_b[:, tsl], pab[32:40, :])
                    nc.vector.tensor_tensor(
                        c_sb[:, tsl], pab[0:R, :], ab_b[:, tsl], AluOpType.mult
                    )
                # y_{t+1} = alpha*y + c_t  (tiny R-space IIR scan)
                y_sb = cyp.tile([R, T], F32, tag="y")
                nc.vector.tensor_tensor_scan(
                    y_sb[:],
                    alpha_sb[0:R, :],
                    c_sb[:],
                    0.0,
                    AluOpType.mult,
                    AluOpType.add,
                )

                # traj = x0scan + L y ; out = tanh(traj) @ W_out^T + b_out
                for tci in range(T // TC):
                    tsl = slice(tci * TC, (tci + 1) * TC)
                    r1s = []
                    for hc in range(NHC):
                        hsl = slice(hc * P, (hc + 1) * P)
                        pt = pst.tile([P, TC], F32)
                        nc.tensor.matmul(
                            pt[:], lt_sb[:, hsl], y_sb[:, tsl], start=True, stop=True
                        )
                        tr = trp.tile([P, TC], F32)
                        nc.vector.tensor_tensor(
                            tr[:], pt[:], xts[hc][:, tsl], AluOpType.add
                        )
                        nc.sync.dma_start(out=traj[b, hsl, tsl], in_=tr[:])
                        if tci == T // TC - 1:
                            nc.sync.dma_start(
                                out=xlast[b, hsl].rearrange("(h o) -> h o", o=1),
                                in_=tr[:, TC - 1 : TC],
                            )
                        r1 = r1p.tile([P, TC], F32)
                        nc.scalar.activation(r1[:], tr[:], TANH)
                        r1s.append(r1)
                    po = pso.tile([O, TC], F32)
                    nc.tensor.matmul(
                        po[:], bout_sb[:], ones_sb[:], start=True, stop=False
                    )
                    for hc in range(NHC):
                        nc.tensor.matmul(
                            po[:],
                            woutt_sb[:, hc, :],
                            r1s[hc][:],
                            start=False,
                            stop=(hc == NHC - 1),
                        )
                    ot = otp.tile([O, TC], F32)
                    nc.scalar.copy(ot[:], po[:])
                    nc.sync.dma_start(out=outp[b, :, tsl], in_=ot[:])

    nc.finalize()
    return nc


_NC_CACHE = None
_LAST_EXEC_NS = None
_LAST_RESULTS = None


def _get_nc():
    global _NC_CACHE
    if _NC_CACHE is None:
        _NC_CACHE = _build_nc()
    return _NC_CACHE


def kernel(u, x0, noise, L, M, N, W_in, b_in, W_out, b_out):
    u = np.asarray(u, np.float32)
    x0 = np.asarray(x0, np.float32)
    noise = np.asarray(noise, np.float32)
    L = np.asarray(L, np.float32)
    M = np.asarray(M, np.float32)
    N = np.asarray(N, np.float32)
    W_in = np.asarray(W_in, np.float32)
    b_in = np.asarray(b_in, np.float32)
    W_out = np.asarray(W_out, np.float32)
    b_out = np.asarray(b_out, np.float32)

    # host-side constant prep (replicated across cores)
    wint = np.concatenate(
        [(TAU * W_in).T.astype(np.float32), (TAU * b_in)[None, :].astype(np.float32)],
        axis=0,
    )  # [I+1, H]
    mn = np.zeros((H, 40), dtype=ml_dtypes.bfloat16)
    mn[:, 0:R] = (KAPPA * M).astype(ml_dtypes.bfloat16)
    mn[:, 32 : 32 + R] = N.astype(ml_dtypes.bfloat16)
    lt = np.ascontiguousarray(L.T).astype(np.float32)  # [R, H]
    woutt = np.ascontiguousarray(W_out.T).astype(np.float32)  # [H, O]
    boutr = np.ascontiguousarray(b_out[None, :]).astype(np.float32)  # [1, O]
    identstd = (NOISE_STD * np.eye(P)).astype(np.float32)

    in_maps = []
    for k in range(NCORES):
        bsl = slice(k * BP, (k + 1) * BP)
        in_maps.append(
            {
                "u": np.ascontiguousarray(u[bsl].transpose(0, 2, 1)),
                "noise": np.ascontiguousarray(noise[:, bsl, :].transpose(1, 2, 0)),
                "x0t": np.ascontiguousarray(x0[bsl].T),
                "wint": wint,
                "mn": mn,
                "lt": lt,
                "woutt": woutt,
                "bout": boutr,
                "identstd": identstd,
            }
        )

    nc = _get_nc()
    import os

    trace = bool(int(os.environ.get("KERNEL_TRACE", "0")))
    res = run_bass_kernel_spmd(nc, in_maps, core_ids=list(range(NCORES)), trace=trace)
    global _LAST_EXEC_NS, _LAST_RESULTS
    _LAST_EXEC_NS = res.exec_time_ns
    _LAST_RESULTS = res

    out = np.ascontiguousarray(
        np.concatenate([res.results[k]["outp"] for k in range(NCORES)], axis=0)
        .transpose(0, 2, 1)
    )
    x_last = np.concatenate([res.results[k]["xlast"] for k in range(NCORES)], axis=0)
    trj = np.ascontiguousarray(
        np.concatenate([res.results[k]["traj"] for k in range(NCORES)], axis=0)
        .transpose(0, 2, 1)
    )
    return out, x_last, trj
